# revision 5
# baseline (speedup 1.0000x reference)
"""Trainium2 Bass kernel for a BFP-quantized ResNet BasicBlock (inference).

Computes, per image (NCHW, C=128, H=W=56):
    out = relu( bn2( conv3x3( q( relu(bn1( conv3x3(q(x), q(w1)) )) ), q(w2)) ) + x )
where q() is HBFP block-floating-point quantization: blocks of 64 contiguous
values (in flat row-major order) share a power-of-2 scale 2^(floor(log2(max|x|))-7),
mantissas rounded (RNE) to 8 signed bits and clamped to +-127.

Key facts exploited:
  * Quantized values are (int in [-127,127]) * 2^k  -> exactly representable in
    bf16, so convs run on the PE at bf16 speed with zero extra error.
  * floor(log2(m)) for normal floats == exponent-field extraction (bitwise ops).
  * RNE rounding == (t + 1.5*2^23) - 1.5*2^23 in fp32 (one dual-op tensor_scalar).
  * clip(round(t)) == round(clamp(t, -127.4, 127.4)) elementwise.
  * conv3x3 = 9 accumulated matmuls (C_in=128 on partitions) over a zero-padded
    58-pitch image layout, fully contiguous rhs slices of 464 columns (8 rows).

Pipeline (v2): the quantized image is inserted into the zero-padded conv input
layout with a ScalarE strided copy (NOT a DMA — the 58-pitch row DMA fragments
into 112B packets that swamp all 16 DMA engines and inflate every DVE op via
SBUF contention).  Stages are software-pipelined ~2 images deep so the
quant-chain latency (V->G->S, ~15us) hides behind PE conv work (~25us/image):

    iter k emits: conv2(k) | quant2(k+1) | conv1(k+2) | quant1(k+3) |
                  final(k-1) | load_x(k+4)

Sharding: data-parallel over batch N=64 -> 8 images per NeuronCore, weights and
BN constants replicated. All 8 cores run the same NEFF (SPMD).
"""

import os

os.environ.setdefault("MYCRO_LOCAL_CACHE", "1")

from contextlib import ExitStack
from functools import lru_cache

import numpy as np

import concourse.bass as bass
import concourse.tile as tile
from concourse import bacc, mybir
from concourse.bass_utils import run_bass_kernel_spmd
from concourse.masks import make_identity

P = 128
H = W = 56
HWF = H * W            # 3136 flat pixels per channel
NBX = HWF // 64        # 49 BFP blocks per channel image
WLEN = 128 * 9         # 1152 flat weight row per output channel
NBW = WLEN // 64       # 18 BFP blocks per weight row
PITCH = W + 2          # 58 padded row pitch
PADLEN = PITCH * PITCH + 2  # 3366: [1 pre-pad][58x58 padded image][1 post-pad]
NCHUNK = 7             # 8-row output chunks per image
CHF = 8 * PITCH        # 464 matmul free dim per chunk
CROUND = 12582912.0    # 1.5 * 2**23  (RNE magic constant)
CLIPV = 127.4
EXPMASK = 0x7F800000
BIAS7 = 7 << 23
C254 = 254 << 23
EGUARD = 50 << 23      # exponent field of 1e-23 (reference's zero-guard)
BN_EPS = 1e-5

F32 = mybir.dt.float32
BF16 = mybir.dt.bfloat16
I32 = mybir.dt.int32
ALU = mybir.AluOpType
ACTF = mybir.ActivationFunctionType
AX = mybir.AxisListType

N_CORES = 8
NIMG = 8  # images per core
NXRAW = 5  # x-residual buffers in flight


def _interior(pad_tile):
    """[P, 56, 56] strided view (pitch 58) of the padded tile's interior."""
    base = 1 + PITCH + 1  # (h=0, w=0) -> index 1 + (h+1)*58 + (w+1)
    v = pad_tile[:, base : base + H * PITCH]
    return v.rearrange("p (h w) -> p h w", w=PITCH)[:, :, :W]


def _emit_scale_bits(nc, small, src3, nb, tag):
    """Per-block scale/rscale from src3 (f32 [P, nb, 64]); all on VectorE.

    Returns (rb, sc): rscale as I32 bits tile (bitcast to f32 for use) and
    scale as BF16 tile.
    """
    bm = small.tile([P, nb], F32, tag=f"bm_{tag}")
    sb = small.tile([P, nb], I32, tag=f"sb_{tag}")
    rb = small.tile([P, nb], I32, tag=f"rb_{tag}")
    sc = small.tile([P, nb], BF16, tag=f"sc_{tag}")
    nc.vector.tensor_reduce(
        out=bm[:], in_=src3, axis=AX.X, op=ALU.max, apply_absolute_value=True,
    )
    # scale bits = max(exponent field, expfield(1e-23)) - (7 << 23)
    # (the max reproduces the reference's +1e-23 zero-guard)
    nc.vector.tensor_scalar(sb[:], bm[:].bitcast(I32), EXPMASK, None,
                            ALU.bitwise_and)
    nc.vector.tensor_scalar(sb[:], sb[:], EGUARD, BIAS7, ALU.max, ALU.subtract)
    # rscale bits = (254 << 23) - scale_bits  -> rscale = 2^(7-e) = 1/scale
    nc.vector.tensor_scalar(rb[:], sb[:], C254, -1, ALU.subtract, ALU.mult)
    nc.vector.tensor_copy(sc[:], sb[:].bitcast(F32))
    return rb, sc


def build_nc(nimg=NIMG):
    nc = bacc.Bacc("TRN2", target_bir_lowering=False, debug=False,
                   enable_asserts=False)

    x_d = nc.dram_tensor("x", [nimg, P, H, W], F32, kind="ExternalInput").ap()
    w1_d = nc.dram_tensor("w1", [P, P, 3, 3], F32, kind="ExternalInput").ap()
    w2_d = nc.dram_tensor("w2", [P, P, 3, 3], F32, kind="ExternalInput").ap()
    bn_d = {
        name: nc.dram_tensor(name, [P], F32, kind="ExternalInput").ap()
        for name in ("gamma1", "beta1", "mean1", "var1",
                     "gamma2", "beta2", "mean2", "var2")
    }
    out_d = nc.dram_tensor("out", [nimg, P, H, W], F32, kind="ExternalOutput").ap()

    with tile.TileContext(nc) as tc, ExitStack() as ctx:
        const = ctx.enter_context(tc.tile_pool(name="const", bufs=1))
        small = ctx.enter_context(tc.tile_pool(name="small", bufs=3))
        xraw_p = ctx.enter_context(tc.tile_pool(name="xraw", bufs=NXRAW))

        # ---- input loads first so HBM DMA overlaps the weight setup ----
        xraws = [None] * nimg

        def load_x(n):
            xraw = xraw_p.tile([P, HWF], F32, tag="xraw", name=f"xraw{n}")
            xraws[n] = xraw
            nc.sync.dma_start(xraw[:], x_d[n].rearrange("c h w -> c (h w)"))

        # weight + BN-constant DMAs
        with tc.tile_pool(name="setup", bufs=1) as setup, \
             tc.tile_pool(name="psum_setup", bufs=2, space="PSUM") as psum_setup:
            wraws = []
            for wi, w_d in enumerate((w1_d, w2_d)):
                wraw = setup.tile([P, WLEN], F32, tag=f"wraw{wi}")
                nc.sync.dma_start(wraw[:], w_d.rearrange("o i kh kw -> o (i kh kw)"))
                wraws.append(wraw)
            bnc = {}
            for name in ("gamma1", "beta1", "mean1", "var1",
                         "gamma2", "beta2", "mean2", "var2"):
                t = setup.tile([P, 1], F32, tag=f"bn_{name}")
                nc.sync.dma_start(t[:], bn_d[name][:, None])
                bnc[name] = t
            for n in range(min(4, nimg)):
                load_x(n)

            # ---- weights quant + transpose, BN constants ----
            ident = const.tile([P, P], BF16, tag="ident")
            make_identity(nc, ident[:])
            zero_b = const.tile([P, 1], F32, tag="zero_b")
            nc.vector.memset(zero_b[:], 0.0)
            eps_b = const.tile([P, 1], F32, tag="eps_b")
            nc.vector.memset(eps_b[:], BN_EPS)

            wks = []
            for wi, wraw in enumerate(wraws):
                wsrc3 = wraw[:].rearrange("p (b e) -> p b e", e=64)
                rb, sc = _emit_scale_bits(nc, small, wsrc3, NBW, f"w{wi}")
                wt_t = setup.tile([P, WLEN], F32, tag=f"wt{wi}")
                wt3 = wt_t[:].rearrange("p (b e) -> p b e", e=64)
                rsc = rb[:].bitcast(F32)[:, :, None].to_broadcast((P, NBW, 64))
                nc.vector.tensor_tensor(wt3, wsrc3, rsc, ALU.mult)
                nc.vector.tensor_scalar(wt_t[:], wt_t[:], CLIPV, -CLIPV,
                                        ALU.min, ALU.max)
                wq = setup.tile([P, WLEN], BF16, tag=f"wq{wi}")
                nc.vector.tensor_scalar(wq[:], wt_t[:], CROUND, CROUND,
                                        ALU.add, ALU.subtract)
                scb = sc[:][:, :, None].to_broadcast((P, NBW, 64))
                wq3 = wq[:].rearrange("p (b e) -> p b e", e=64)
                nc.vector.tensor_tensor(wq3, wq3, scb, ALU.mult)
                # per-offset lhsT tiles: w[k][i, o] = wq[o, i*9+k]
                wq_v = wq[:].rearrange("p (i k) -> p k i", k=9)
                wk = []
                for k in range(9):
                    pt = psum_setup.tile([P, P], BF16, tag="tps")
                    nc.tensor.transpose(pt[:], wq_v[:, k, :], ident[:])
                    wt = const.tile([P, P], BF16, tag=f"w{wi}k{k}")
                    nc.scalar.copy(wt[:], pt[:])
                    wk.append(wt)
                wks.append(wk)
            w1k, w2k = wks

            invb = []
            for i in ("1", "2"):
                s = setup.tile([P, 1], F32, tag=f"sd{i}")
                nc.scalar.activation(s[:], bnc[f"var{i}"][:], ACTF.Sqrt, bias=eps_b[:])
                r = setup.tile([P, 1], F32, tag=f"rs{i}")
                nc.vector.reciprocal(r[:], s[:])
                inv = const.tile([P, 1], F32, tag=f"inv{i}")
                nc.vector.tensor_tensor(inv[:], bnc[f"gamma{i}"][:], r[:], ALU.mult)
                mi = setup.tile([P, 1], F32, tag=f"mi{i}")
                nc.vector.tensor_tensor(mi[:], bnc[f"mean{i}"][:], inv[:], ALU.mult)
                b = const.tile([P, 1], F32, tag=f"b{i}")
                nc.vector.tensor_tensor(b[:], bnc[f"beta{i}"][:], mi[:], ALU.subtract)
                invb.append((inv, b))
            (inv1, b1), (inv2, b2) = invb

        pads = ctx.enter_context(tc.tile_pool(name="pads", bufs=1))
        tmp_p = ctx.enter_context(tc.tile_pool(name="tmp", bufs=2))
        u_p = ctx.enter_context(tc.tile_pool(name="u", bufs=2))
        mid_p = ctx.enter_context(tc.tile_pool(name="mid", bufs=2))
        t2_p = ctx.enter_context(tc.tile_pool(name="t2", bufs=2))

        xq_pads = [pads.tile([P, PADLEN], BF16, tag=f"xqp{i}", name=f"xqp{i}")
                   for i in range(2)]
        mq_pads = [pads.tile([P, PADLEN], BF16, tag=f"mqp{i}", name=f"mqp{i}")
                   for i in range(2)]
        for t in (*xq_pads, *mq_pads):
            nc.vector.memset(t[:], 0.0)

        psum1_p = ctx.enter_context(tc.tile_pool(name="psum1", bufs=4, space="PSUM"))
        psum2_p = ctx.enter_context(tc.tile_pool(name="psum2", bufs=4, space="PSUM"))

        mids = [None] * nimg
        t2s = [None] * nimg

        def quant1(n):
            """x -> quantized bf16 -> xq_pad.  V: reduce/smalls/mult;
            G: clip/round/scale; S: strided copy into pad interior."""
            xr3 = xraws[n][:].rearrange("p (b e) -> p b e", e=64)
            rb, sc = _emit_scale_bits(nc, small, xr3, NBX, "q1")
            t = tmp_p.tile([P, HWF], F32, tag="t", name=f"t1_{n}")
            t3 = t[:].rearrange("p (b e) -> p b e", e=64)
            rsc = rb[:].bitcast(F32)[:, :, None].to_broadcast((P, NBX, 64))
            nc.vector.tensor_tensor(t3, xr3, rsc, ALU.mult)
            nc.gpsimd.tensor_scalar(t[:], t[:], CLIPV, -CLIPV, ALU.min, ALU.max)
            u = u_p.tile([P, HWF], BF16, tag="u", name=f"u1_{n}")
            nc.gpsimd.tensor_scalar(u[:], t[:], CROUND, CROUND,
                                    ALU.add, ALU.subtract)
            scb = sc[:][:, :, None].to_broadcast((P, NBX, 64))
            u3 = u[:].rearrange("p (b e) -> p b e", e=64)
            nc.gpsimd.tensor_tensor(u3, u3, scb, ALU.mult)
            nc.scalar.copy(_interior(xq_pads[n % 2]),
                           u[:].rearrange("p (h w) -> p h w", w=W))

        def quant2(n):
            """mid -> quantized bf16 -> mq_pad.  V: everything but the
            scale-back (G) and pad insert (S)."""
            mid3 = mids[n][:].rearrange("p (b e) -> p b e", e=64)
            rb, sc = _emit_scale_bits(nc, small, mid3, NBX, "q2")
            t = tmp_p.tile([P, HWF], F32, tag="t", name=f"t2q_{n}")
            t3 = t[:].rearrange("p (b e) -> p b e", e=64)
            rsc = rb[:].bitcast(F32)[:, :, None].to_broadcast((P, NBX, 64))
            nc.vector.tensor_tensor(t3, mid3, rsc, ALU.mult)
            nc.vector.tensor_scalar(t[:], t[:], CLIPV, -CLIPV, ALU.min, ALU.max)
            u = u_p.tile([P, HWF], BF16, tag="u", name=f"u2_{n}")
            nc.vector.tensor_scalar(u[:], t[:], CROUND, CROUND,
                                    ALU.add, ALU.subtract)
            scb = sc[:][:, :, None].to_broadcast((P, NBX, 64))
            u3 = u[:].rearrange("p (b e) -> p b e", e=64)
            nc.gpsimd.tensor_tensor(u3, u3, scb, ALU.mult)
            nc.scalar.copy(_interior(mq_pads[n % 2]),
                           u[:].rearrange("p (h w) -> p h w", w=W))

        def _emit_conv(psum_pool, wk, src_pad, evict):
            for c in range(NCHUNK):
                h0 = c * 8
                ps = psum_pool.tile([P, CHF], F32, tag="pschunk")
                for k in range(9):
                    kh, kw = divmod(k, 3)
                    s = (h0 + kh) * PITCH + kw
                    nc.tensor.matmul(
                        ps[:], wk[k][:], src_pad[:, s : s + CHF],
                        start=(k == 0), stop=(k == 8),
                    )
                evict(c, ps)

        def conv1(n):
            mid = mid_p.tile([P, HWF], F32, tag="mid", name=f"mid{n}")
            mids[n] = mid

            def evict1(c, ps):
                psv = ps[:].rearrange("p (r w) -> p r w", w=PITCH)[:, :, 1 : 1 + W]
                ov = mid[:, c * 448 : (c + 1) * 448].rearrange("p (r w) -> p r w", w=W)
                nc.scalar.activation(ov, psv, ACTF.Relu, bias=b1[:], scale=inv1[:])

            _emit_conv(psum1_p, w1k, xq_pads[n % 2][:], evict1)

        def conv2(n):
            t2 = t2_p.tile([P, HWF], F32, tag="t2", name=f"t2_{n}")
            t2s[n] = t2

            def evict2(c, ps):
                psv = ps[:].rearrange("p (r w) -> p r w", w=PITCH)[:, :, 1 : 1 + W]
                ov = t2[:, c * 448 : (c + 1) * 448].rearrange("p (r w) -> p r w", w=W)
                nc.scalar.activation(ov, psv, ACTF.Identity, bias=b2[:], scale=inv2[:])

            _emit_conv(psum2_p, w2k, mq_pads[n % 2][:], evict2)

        def final(n):
            t2 = t2s[n]
            nc.vector.tensor_tensor(t2[:], t2[:], xraws[n][:], ALU.add)
            nc.scalar.activation(t2[:], t2[:], ACTF.Relu, bias=zero_b[:])
            nc.sync.dma_start(out_d[n].rearrange("c h w -> c (h w)"), t2[:])

        # ---- software pipeline ----
        quant1(0)
        quant1(1)
        conv1(0)
        if nimg > 2:
            quant1(2)   # after conv1(0): overwrites xq_pads[0]
        quant2(0)
        if nimg > 1:
            conv1(1)
        for k in range(nimg):
            conv2(k)
            if k + 1 < nimg:
                quant2(k + 1)
            if k + 2 < nimg:
                conv1(k + 2)
            if k + 3 < nimg:
                quant1(k + 3)
            if k >= 1:
                final(k - 1)
            if k + 4 < nimg:
                load_x(k + 4)
        final(nimg - 1)

    nc.compile()
    return nc


@lru_cache(maxsize=1)
def _get_nc():
    return build_nc(NIMG)


def kernel(x, w1, w2, gamma1, beta1, mean1, var1,
           gamma2, beta2, mean2, var2, _trace=False):
    f = lambda a: np.ascontiguousarray(np.asarray(a, dtype=np.float32))
    x = f(x)
    n_total = x.shape[0]
    assert n_total == N_CORES * NIMG, x.shape
    xs = x.reshape(N_CORES, NIMG, P, H, W)
    rep = {
        "w1": f(w1), "w2": f(w2),
        "gamma1": f(gamma1), "beta1": f(beta1), "mean1": f(mean1), "var1": f(var1),
        "gamma2": f(gamma2), "beta2": f(beta2), "mean2": f(mean2), "var2": f(var2),
    }
    in_maps = [{"x": np.ascontiguousarray(xs[c]), **rep} for c in range(N_CORES)]
    nc = _get_nc()
    res = run_bass_kernel_spmd(nc, in_maps, core_ids=list(range(N_CORES)),
                               trace=_trace)
    out = np.concatenate([res.results[c]["out"] for c in range(N_CORES)], axis=0)
    if _trace:
        kernel.last_result = res
    return out.reshape(n_total, P, H, W)


# revision 6
# speedup vs baseline: 1.9403x; 1.9403x over previous
"""Trainium2 Bass kernel for a BFP-quantized ResNet BasicBlock (inference).

Computes, per image (NCHW, C=128, H=W=56):
    out = relu( bn2( conv3x3( q( relu(bn1( conv3x3(q(x), q(w1)) )) ), q(w2)) ) + x )
where q() is HBFP block-floating-point quantization: blocks of 64 contiguous
values (in flat row-major order) share a power-of-2 scale 2^(floor(log2(max|x|))-7),
mantissas rounded (RNE) to 8 signed bits and clamped to +-127.

Key facts exploited:
  * Quantized values are (int in [-127,127]) * 2^k  -> exactly representable in
    bf16, so convs run on the PE at bf16 speed with zero extra error.
  * floor(log2(m)) for normal floats == exponent-field extraction (bitwise ops).
  * RNE rounding == (t + 1.5*2^23) - 1.5*2^23 in fp32 (one dual-op tensor_scalar).
  * clip(round(t)) == round(clamp(t, -127.4, 127.4)) elementwise.
  * conv3x3 = 9 accumulated matmuls (C_in=128 on partitions) over a zero-padded
    58-pitch image layout, fully contiguous rhs slices of 464 columns (8 rows).

Pipeline (v2): the quantized image is inserted into the zero-padded conv input
layout with a ScalarE strided copy (NOT a DMA — the 58-pitch row DMA fragments
into 112B packets that swamp all 16 DMA engines and inflate every DVE op via
SBUF contention).  Stages are software-pipelined ~2 images deep so the
quant-chain latency (V->G->S, ~15us) hides behind PE conv work (~25us/image):

    iter k emits: conv2(k) | quant2(k+1) | conv1(k+2) | quant1(k+3) |
                  final(k-1) | load_x(k+4)

Sharding: data-parallel over batch N=64 -> 8 images per NeuronCore, weights and
BN constants replicated. All 8 cores run the same NEFF (SPMD).
"""

import os

os.environ.setdefault("MYCRO_LOCAL_CACHE", "1")

from contextlib import ExitStack
from functools import lru_cache

import numpy as np

import concourse.bass as bass
import concourse.tile as tile
from concourse import bacc, mybir
from concourse.bass_utils import run_bass_kernel_spmd
from concourse.masks import make_identity

P = 128
H = W = 56
HWF = H * W            # 3136 flat pixels per channel
NBX = HWF // 64        # 49 BFP blocks per channel image
WLEN = 128 * 9         # 1152 flat weight row per output channel
NBW = WLEN // 64       # 18 BFP blocks per weight row
PITCH = W + 2          # 58 padded row pitch
PADLEN = PITCH * PITCH + 2  # 3366: [1 pre-pad][58x58 padded image][1 post-pad]
NCHUNK = 7             # 8-row output chunks per image
CHF = 8 * PITCH        # 464 matmul free dim per chunk
CROUND = 12582912.0    # 1.5 * 2**23  (RNE magic constant)
CLIPV = 127.4
EXPMASK = 0x7F800000
BIAS7 = 7 << 23
C254 = 254 << 23
EGUARD = 50 << 23      # exponent field of 1e-23 (reference's zero-guard)
BN_EPS = 1e-5

F32 = mybir.dt.float32
BF16 = mybir.dt.bfloat16
I32 = mybir.dt.int32
ALU = mybir.AluOpType
ACTF = mybir.ActivationFunctionType
AX = mybir.AxisListType

N_CORES = 8
NIMG = 8  # images per core
NXRAW = 5  # x-residual buffers in flight


def _interior(pad_tile):
    """[P, 56, 56] strided view (pitch 58) of the padded tile's interior."""
    base = 1 + PITCH + 1  # (h=0, w=0) -> index 1 + (h+1)*58 + (w+1)
    v = pad_tile[:, base : base + H * PITCH]
    return v.rearrange("p (h w) -> p h w", w=PITCH)[:, :, :W]


def _emit_scale_bits(nc, small, src3, nb, tag):
    """Per-block scale/rscale from src3 (f32 [P, nb, 64]); all on VectorE.

    Returns (rb, sc): rscale as I32 bits tile (bitcast to f32 for use) and
    scale as BF16 tile.
    """
    bm = small.tile([P, nb], F32, tag=f"bm_{tag}")
    sb = small.tile([P, nb], I32, tag=f"sb_{tag}")
    rb = small.tile([P, nb], I32, tag=f"rb_{tag}")
    sc = small.tile([P, nb], BF16, tag=f"sc_{tag}")
    nc.vector.tensor_reduce(
        out=bm[:], in_=src3, axis=AX.X, op=ALU.max, apply_absolute_value=True,
    )
    # scale bits = max(exponent field, expfield(1e-23)) - (7 << 23)
    # (the max reproduces the reference's +1e-23 zero-guard)
    nc.vector.tensor_scalar(sb[:], bm[:].bitcast(I32), EXPMASK, None,
                            ALU.bitwise_and)
    nc.vector.tensor_scalar(sb[:], sb[:], EGUARD, BIAS7, ALU.max, ALU.subtract)
    # rscale bits = (254 << 23) - scale_bits  -> rscale = 2^(7-e) = 1/scale
    nc.vector.tensor_scalar(rb[:], sb[:], C254, -1, ALU.subtract, ALU.mult)
    nc.vector.tensor_copy(sc[:], sb[:].bitcast(F32))
    return rb, sc


def build_nc(nimg=NIMG):
    nc = bacc.Bacc("TRN2", target_bir_lowering=False, debug=False,
                   enable_asserts=False)

    x_d = nc.dram_tensor("x", [nimg, P, H, W], F32, kind="ExternalInput").ap()
    w1_d = nc.dram_tensor("w1", [P, P, 3, 3], F32, kind="ExternalInput").ap()
    w2_d = nc.dram_tensor("w2", [P, P, 3, 3], F32, kind="ExternalInput").ap()
    bn_d = {
        name: nc.dram_tensor(name, [P], F32, kind="ExternalInput").ap()
        for name in ("gamma1", "beta1", "mean1", "var1",
                     "gamma2", "beta2", "mean2", "var2")
    }
    out_d = nc.dram_tensor("out", [nimg, P, H, W], F32, kind="ExternalOutput").ap()

    with tile.TileContext(nc) as tc, ExitStack() as ctx:
        const = ctx.enter_context(tc.tile_pool(name="const", bufs=1))
        small = ctx.enter_context(tc.tile_pool(name="small", bufs=3))
        xraw_p = ctx.enter_context(tc.tile_pool(name="xraw", bufs=NXRAW))

        # ---- input loads first so HBM DMA overlaps the weight setup ----
        xraws = [None] * nimg

        def load_x(n):
            xraw = xraw_p.tile([P, HWF], F32, tag="xraw", name=f"xraw{n}")
            xraws[n] = xraw
            nc.sync.dma_start(xraw[:], x_d[n].rearrange("c h w -> c (h w)"))

        # weight + BN-constant DMAs
        with tc.tile_pool(name="setup", bufs=1) as setup, \
             tc.tile_pool(name="psum_setup", bufs=2, space="PSUM") as psum_setup:
            wraws = []
            for wi, w_d in enumerate((w1_d, w2_d)):
                wraw = setup.tile([P, WLEN], F32, tag=f"wraw{wi}")
                nc.sync.dma_start(wraw[:], w_d.rearrange("o i kh kw -> o (i kh kw)"))
                wraws.append(wraw)
            bnc = {}
            for name in ("gamma1", "beta1", "mean1", "var1",
                         "gamma2", "beta2", "mean2", "var2"):
                t = setup.tile([P, 1], F32, tag=f"bn_{name}")
                nc.sync.dma_start(t[:], bn_d[name][:, None])
                bnc[name] = t
            for n in range(min(4, nimg)):
                load_x(n)

            # ---- weights quant + transpose, BN constants ----
            ident = const.tile([P, P], BF16, tag="ident")
            make_identity(nc, ident[:])
            zero_b = const.tile([P, 1], F32, tag="zero_b")
            nc.vector.memset(zero_b[:], 0.0)
            eps_b = const.tile([P, 1], F32, tag="eps_b")
            nc.vector.memset(eps_b[:], BN_EPS)

            wks = []
            for wi, wraw in enumerate(wraws):
                wsrc3 = wraw[:].rearrange("p (b e) -> p b e", e=64)
                rb, sc = _emit_scale_bits(nc, small, wsrc3, NBW, f"w{wi}")
                wt_t = setup.tile([P, WLEN], F32, tag=f"wt{wi}")
                wt3 = wt_t[:].rearrange("p (b e) -> p b e", e=64)
                rsc = rb[:].bitcast(F32)[:, :, None].to_broadcast((P, NBW, 64))
                nc.vector.tensor_tensor(wt3, wsrc3, rsc, ALU.mult)
                nc.vector.tensor_scalar(wt_t[:], wt_t[:], CLIPV, -CLIPV,
                                        ALU.min, ALU.max)
                wq = setup.tile([P, WLEN], BF16, tag=f"wq{wi}")
                nc.vector.tensor_scalar(wq[:], wt_t[:], CROUND, CROUND,
                                        ALU.add, ALU.subtract)
                scb = sc[:][:, :, None].to_broadcast((P, NBW, 64))
                wq3 = wq[:].rearrange("p (b e) -> p b e", e=64)
                nc.vector.tensor_tensor(wq3, wq3, scb, ALU.mult)
                # per-offset lhsT tiles: w[k][i, o] = wq[o, i*9+k]
                wq_v = wq[:].rearrange("p (i k) -> p k i", k=9)
                wk = []
                for k in range(9):
                    pt = psum_setup.tile([P, P], BF16, tag="tps")
                    nc.tensor.transpose(pt[:], wq_v[:, k, :], ident[:])
                    wt = const.tile([P, P], BF16, tag=f"w{wi}k{k}")
                    nc.scalar.copy(wt[:], pt[:])
                    wk.append(wt)
                wks.append(wk)
            w1k, w2k = wks

            invb = []
            for i in ("1", "2"):
                s = setup.tile([P, 1], F32, tag=f"sd{i}")
                nc.scalar.activation(s[:], bnc[f"var{i}"][:], ACTF.Sqrt, bias=eps_b[:])
                r = setup.tile([P, 1], F32, tag=f"rs{i}")
                nc.vector.reciprocal(r[:], s[:])
                inv = const.tile([P, 1], F32, tag=f"inv{i}")
                nc.vector.tensor_tensor(inv[:], bnc[f"gamma{i}"][:], r[:], ALU.mult)
                mi = setup.tile([P, 1], F32, tag=f"mi{i}")
                nc.vector.tensor_tensor(mi[:], bnc[f"mean{i}"][:], inv[:], ALU.mult)
                b = const.tile([P, 1], F32, tag=f"b{i}")
                nc.vector.tensor_tensor(b[:], bnc[f"beta{i}"][:], mi[:], ALU.subtract)
                invb.append((inv, b))
            (inv1, b1), (inv2, b2) = invb

        pads = ctx.enter_context(tc.tile_pool(name="pads", bufs=1))
        tmp_p = ctx.enter_context(tc.tile_pool(name="tmp", bufs=2))
        u_p = ctx.enter_context(tc.tile_pool(name="u", bufs=2))
        mid_p = ctx.enter_context(tc.tile_pool(name="mid", bufs=2))
        t2_p = ctx.enter_context(tc.tile_pool(name="t2", bufs=2))

        xq_pads = [pads.tile([P, PADLEN], BF16, tag=f"xqp{i}", name=f"xqp{i}")
                   for i in range(2)]
        mq_pads = [pads.tile([P, PADLEN], BF16, tag=f"mqp{i}", name=f"mqp{i}")
                   for i in range(2)]
        for t in (*xq_pads, *mq_pads):
            nc.vector.memset(t[:], 0.0)

        psum1_p = ctx.enter_context(tc.tile_pool(name="psum1", bufs=4, space="PSUM"))
        psum2_p = ctx.enter_context(tc.tile_pool(name="psum2", bufs=4, space="PSUM"))

        mids = [None] * nimg
        t2s = [None] * nimg

        def _quant_image(src, pad, tagq, nm):
            """BFP-quantize src (f32 [P,3136]) into pad's interior.

            Engine split respects the DVE/GpSimd shared-SBUF-port rule:
            V does reduce + TT mult (single-port) and clip/round (2-port TS);
            G does ONLY the scale-back TT, emitted so it overlaps the next
            V single-port phase; S inserts into the padded layout (own ports).
            """
            src3 = src[:].rearrange("p (b e) -> p b e", e=64)
            rb, sc = _emit_scale_bits(nc, small, src3, NBX, tagq)
            t = tmp_p.tile([P, HWF], F32, tag="t", name=f"t_{nm}")
            t3 = t[:].rearrange("p (b e) -> p b e", e=64)
            rsc = rb[:].bitcast(F32)[:, :, None].to_broadcast((P, NBX, 64))
            nc.vector.tensor_tensor(t3, src3, rsc, ALU.mult)
            nc.vector.tensor_scalar(t[:], t[:], CLIPV, -CLIPV, ALU.min, ALU.max)
            u = u_p.tile([P, HWF], BF16, tag="u", name=f"u_{nm}")
            nc.vector.tensor_scalar(u[:], t[:], CROUND, CROUND,
                                    ALU.add, ALU.subtract)
            scb = sc[:][:, :, None].to_broadcast((P, NBX, 64))
            u3 = u[:].rearrange("p (b e) -> p b e", e=64)
            nc.gpsimd.tensor_tensor(u3, u3, scb, ALU.mult)
            nc.scalar.copy(_interior(pad),
                           u[:].rearrange("p (h w) -> p h w", w=W))

        def quant1(n):
            _quant_image(xraws[n], xq_pads[n % 2], "q1", f"q1_{n}")

        def quant2(n):
            _quant_image(mids[n], mq_pads[n % 2], "q2", f"q2_{n}")

        def _emit_conv(psum_pool, wk, src_pad, evict):
            for c in range(NCHUNK):
                h0 = c * 8
                ps = psum_pool.tile([P, CHF], F32, tag="pschunk")
                for k in range(9):
                    kh, kw = divmod(k, 3)
                    s = (h0 + kh) * PITCH + kw
                    nc.tensor.matmul(
                        ps[:], wk[k][:], src_pad[:, s : s + CHF],
                        start=(k == 0), stop=(k == 8),
                    )
                evict(c, ps)

        def conv1(n):
            mid = mid_p.tile([P, HWF], F32, tag="mid", name=f"mid{n}")
            mids[n] = mid

            def evict1(c, ps):
                psv = ps[:].rearrange("p (r w) -> p r w", w=PITCH)[:, :, 1 : 1 + W]
                ov = mid[:, c * 448 : (c + 1) * 448].rearrange("p (r w) -> p r w", w=W)
                nc.scalar.activation(ov, psv, ACTF.Relu, bias=b1[:], scale=inv1[:])

            _emit_conv(psum1_p, w1k, xq_pads[n % 2][:], evict1)

        def conv2(n):
            t2 = t2_p.tile([P, HWF], F32, tag="t2", name=f"t2_{n}")
            t2s[n] = t2

            def evict2(c, ps):
                psv = ps[:].rearrange("p (r w) -> p r w", w=PITCH)[:, :, 1 : 1 + W]
                ov = t2[:, c * 448 : (c + 1) * 448].rearrange("p (r w) -> p r w", w=W)
                nc.scalar.activation(ov, psv, ACTF.Identity, bias=b2[:], scale=inv2[:])

            _emit_conv(psum2_p, w2k, mq_pads[n % 2][:], evict2)

        def final(n):
            t2 = t2s[n]
            nc.vector.tensor_tensor(t2[:], t2[:], xraws[n][:], ALU.add)
            nc.scalar.activation(t2[:], t2[:], ACTF.Relu, bias=zero_b[:])
            nc.sync.dma_start(out_d[n].rearrange("c h w -> c (h w)"), t2[:])

        # ---- software pipeline ----
        quant1(0)
        quant1(1)
        conv1(0)
        if nimg > 2:
            quant1(2)   # after conv1(0): overwrites xq_pads[0]
        quant2(0)
        if nimg > 1:
            conv1(1)
        for k in range(nimg):
            conv2(k)
            if k + 1 < nimg:
                quant2(k + 1)
            if k + 2 < nimg:
                conv1(k + 2)
            if k + 3 < nimg:
                quant1(k + 3)
            if k >= 1:
                final(k - 1)
            if k + 4 < nimg:
                load_x(k + 4)
        final(nimg - 1)

    nc.compile()
    return nc


@lru_cache(maxsize=1)
def _get_nc():
    return build_nc(NIMG)


def kernel(x, w1, w2, gamma1, beta1, mean1, var1,
           gamma2, beta2, mean2, var2, _trace=False):
    f = lambda a: np.ascontiguousarray(np.asarray(a, dtype=np.float32))
    x = f(x)
    n_total = x.shape[0]
    assert n_total == N_CORES * NIMG, x.shape
    xs = x.reshape(N_CORES, NIMG, P, H, W)
    rep = {
        "w1": f(w1), "w2": f(w2),
        "gamma1": f(gamma1), "beta1": f(beta1), "mean1": f(mean1), "var1": f(var1),
        "gamma2": f(gamma2), "beta2": f(beta2), "mean2": f(mean2), "var2": f(var2),
    }
    in_maps = [{"x": np.ascontiguousarray(xs[c]), **rep} for c in range(N_CORES)]
    nc = _get_nc()
    res = run_bass_kernel_spmd(nc, in_maps, core_ids=list(range(N_CORES)),
                               trace=_trace)
    out = np.concatenate([res.results[c]["out"] for c in range(N_CORES)], axis=0)
    if _trace:
        kernel.last_result = res
    return out.reshape(n_total, P, H, W)


# revision 9
# speedup vs baseline: 1.9570x; 1.0086x over previous
"""Trainium2 Bass kernel for a BFP-quantized ResNet BasicBlock (inference).

Computes, per image (NCHW, C=128, H=W=56):
    out = relu( bn2( conv3x3( q( relu(bn1( conv3x3(q(x), q(w1)) )) ), q(w2)) ) + x )
where q() is HBFP block-floating-point quantization: blocks of 64 contiguous
values (in flat row-major order) share a power-of-2 scale 2^(floor(log2(max|x|))-7),
mantissas rounded (RNE) to 8 signed bits and clamped to +-127.

Key facts exploited:
  * Quantized values are (int in [-127,127]) * 2^k  -> exactly representable in
    bf16, so convs run on the PE at bf16 speed with zero extra error.
  * floor(log2(m)) for normal floats == exponent-field extraction (bitwise ops).
  * RNE rounding == (t + 1.5*2^23) - 1.5*2^23 in fp32 (one dual-op tensor_scalar).
  * clip(round(t)) == round(clamp(t, -127.4, 127.4)) elementwise.
  * conv3x3 = 9 accumulated matmuls (C_in=128 on partitions) over a zero-padded
    58-pitch image layout, fully contiguous rhs slices of 464 columns (8 rows).

Scheduling notes (v4):
  * The quantized image is inserted into the padded conv layout by a ScalarE
    strided copy, NOT a DMA (a 58-pitch DMA fragments into 112B packets that
    swamp all 16 DMA engines and inflate every DVE op via SBUF contention).
  * GpSimd shares its only SBUF port pair with DVE's 2-port modes
    (tensor_scalar/copy/cast); whoever issues first holds an exclusive lock.
    So GpSimd gets only tensor_tensor work (scale-backs, residual adds), split
    into halves so a V 2-port op never blocks behind a long G op, and emitted
    so G occupancy overlaps V's single-port phases (tensor_tensor/reduce never
    contend).
  * Weight transposes run on the DMA crossbar (dma_start_transpose) at startup:
    no PSUM round-trip, no PE/ScalarE involvement.
  * Stages are pipelined ~2 images deep; iter k emits conv2(k) | quant2(k+1) |
    final(k-1) | conv1(k+2) | quant1(k+3) | load_x(k+4).  The last image's
    residual+relu+store is fused per-chunk into conv2's eviction to shorten the
    drain tail.

Sharding: data-parallel over batch N=64 -> 8 images per NeuronCore, weights and
BN constants replicated. All 8 cores run the same NEFF (SPMD).
"""

import os

os.environ.setdefault("MYCRO_LOCAL_CACHE", "1")

from contextlib import ExitStack
from functools import lru_cache

import numpy as np

import concourse.bass as bass
import concourse.tile as tile
from concourse import bacc, mybir
from concourse.bass_utils import run_bass_kernel_spmd

P = 128
H = W = 56
HWF = H * W            # 3136 flat pixels per channel
NBX = HWF // 64        # 49 BFP blocks per channel image
WLEN = 128 * 9         # 1152 flat weight row per output channel
NBW = WLEN // 64       # 18 BFP blocks per weight row
PITCH = W + 2          # 58 padded row pitch
PADLEN = PITCH * PITCH + 2  # 3366: [1 pre-pad][58x58 padded image][1 post-pad]
NCHUNK = 7             # 8-row output chunks per image
CHF = 8 * PITCH        # 464 matmul free dim per chunk
CROUND = 12582912.0    # 1.5 * 2**23  (RNE magic constant)
CLIPV = 127.4
EXPMASK = 0x7F800000
BIAS7 = 7 << 23
C254 = 254 << 23
EGUARD = 50 << 23      # exponent field of 1e-23 (reference's zero-guard)
BN_EPS = 1e-5

F32 = mybir.dt.float32
BF16 = mybir.dt.bfloat16
I32 = mybir.dt.int32
ALU = mybir.AluOpType
ACTF = mybir.ActivationFunctionType
AX = mybir.AxisListType

N_CORES = 8
NIMG = 8   # images per core
NXRAW = 5  # x-residual buffers in flight
HB = 25    # block split point for halved G ops (25+24 = 49)


def _interior(pad_tile):
    """[P, 56, 56] strided view (pitch 58) of the padded tile's interior."""
    base = 1 + PITCH + 1  # (h=0, w=0) -> index 1 + (h+1)*58 + (w+1)
    v = pad_tile[:, base : base + H * PITCH]
    return v.rearrange("p (h w) -> p h w", w=PITCH)[:, :, :W]


def build_nc(nimg=NIMG):
    nc = bacc.Bacc("TRN2", target_bir_lowering=False, debug=False,
                   enable_asserts=False)

    x_d = nc.dram_tensor("x", [nimg, P, H, W], F32, kind="ExternalInput").ap()
    w1_d = nc.dram_tensor("w1", [P, P, 3, 3], F32, kind="ExternalInput").ap()
    w2_d = nc.dram_tensor("w2", [P, P, 3, 3], F32, kind="ExternalInput").ap()
    bn_d = {
        name: nc.dram_tensor(name, [P], F32, kind="ExternalInput").ap()
        for name in ("gamma1", "beta1", "mean1", "var1",
                     "gamma2", "beta2", "mean2", "var2")
    }
    out_d = nc.dram_tensor("out", [nimg, P, H, W], F32, kind="ExternalOutput").ap()

    with tile.TileContext(nc) as tc, ExitStack() as ctx:
        const = ctx.enter_context(tc.tile_pool(name="const", bufs=1))
        small = ctx.enter_context(tc.tile_pool(name="small", bufs=3))
        xraw_p = ctx.enter_context(tc.tile_pool(name="xraw", bufs=NXRAW))

        xraws = [None] * nimg

        def load_x(n):
            xraw = xraw_p.tile([P, HWF], F32, tag="xraw", name=f"xraw{n}")
            xraws[n] = xraw
            nc.sync.dma_start(xraw[:], x_d[n].rearrange("c h w -> c (h w)"))

        def _emit_scale_bits(src3, nb, tag):
            """Per-block scale/rscale from src3 (f32 [P, nb, 64]); VectorE."""
            bm = small.tile([P, nb], F32, tag=f"bm_{tag}")
            sb = small.tile([P, nb], I32, tag=f"sb_{tag}")
            rb = small.tile([P, nb], I32, tag=f"rb_{tag}")
            sc = small.tile([P, nb], BF16, tag=f"sc_{tag}")
            nc.vector.tensor_reduce(
                out=bm[:], in_=src3, axis=AX.X, op=ALU.max,
                apply_absolute_value=True,
            )
            # scale bits = max(exponent field, expfield(1e-23)) - (7 << 23)
            nc.vector.tensor_scalar(sb[:], bm[:].bitcast(I32), EXPMASK, None,
                                    ALU.bitwise_and)
            nc.vector.tensor_scalar(sb[:], sb[:], EGUARD, BIAS7,
                                    ALU.max, ALU.subtract)
            # rscale bits = (254 << 23) - scale_bits -> rscale = 2^(7-e)
            nc.vector.tensor_scalar(rb[:], sb[:], C254, -1,
                                    ALU.subtract, ALU.mult)
            nc.vector.tensor_copy(sc[:], sb[:].bitcast(F32))
            return rb, sc

        # ---- DMAs: x0 first (heads the startup critical path), then w1 ----
        load_x(0)
        if True:
            setup = ctx.enter_context(tc.tile_pool(name="setup", bufs=1))
            wraws = []
            for wi, w_d in enumerate((w1_d, w2_d)):
                wraw = setup.tile([P, WLEN], F32, tag=f"wraw{wi}")
                wraws.append(wraw)
            nc.sync.dma_start(wraws[0][:],
                              w1_d.rearrange("o i kh kw -> o (i kh kw)"))
            for n in range(1, min(4, nimg)):
                load_x(n)
            bnc = {}
            for name in ("gamma1", "beta1", "mean1", "var1",
                         "gamma2", "beta2", "mean2", "var2"):
                t = setup.tile([P, 1], F32, tag=f"bn_{name}")
                nc.sync.dma_start(t[:], bn_d[name][:, None])
                bnc[name] = t
            nc.sync.dma_start(wraws[1][:],
                              w2_d.rearrange("o i kh kw -> o (i kh kw)"))

            zero_b = const.tile([P, 1], F32, tag="zero_b")
            nc.vector.memset(zero_b[:], 0.0)
            eps_b = const.tile([P, 1], F32, tag="eps_b")
            nc.vector.memset(eps_b[:], BN_EPS)

            # padded conv-input tiles; zeroed once on GpSimd (idle at startup)
            pads = ctx.enter_context(tc.tile_pool(name="pads", bufs=1))
            tmp_p = ctx.enter_context(tc.tile_pool(name="tmp", bufs=2))
            u_p = ctx.enter_context(tc.tile_pool(name="u", bufs=2))
            mid_p = ctx.enter_context(tc.tile_pool(name="mid", bufs=2))
            t2_p = ctx.enter_context(tc.tile_pool(name="t2", bufs=2))
            xq_pads = [pads.tile([P, PADLEN], BF16, tag=f"xqp{i}", name=f"xqp{i}")
                       for i in range(2)]
            mq_pads = [pads.tile([P, PADLEN], BF16, tag=f"mqp{i}", name=f"mqp{i}")
                       for i in range(2)]
            for t in (*xq_pads, *mq_pads):
                nc.gpsimd.memset(t[:], 0.0)

            def _setup_w(wi):
                """Quantize weight wi (VectorE) + transpose via DMA crossbar."""
                wraw = wraws[wi]
                wsrc3 = wraw[:].rearrange("p (b e) -> p b e", e=64)
                rb, sc = _emit_scale_bits(wsrc3, NBW, f"w{wi}")
                wt_t = setup.tile([P, WLEN], F32, tag="wt")
                wt3 = wt_t[:].rearrange("p (b e) -> p b e", e=64)
                rsc = rb[:].bitcast(F32)[:, :, None].to_broadcast((P, NBW, 64))
                nc.vector.tensor_tensor(wt3, wsrc3, rsc, ALU.mult)
                nc.vector.tensor_scalar(wt_t[:], wt_t[:], CLIPV, -CLIPV,
                                        ALU.min, ALU.max)
                wq = setup.tile([P, WLEN], BF16, tag=f"wq{wi}")
                nc.vector.tensor_scalar(wq[:], wt_t[:], CROUND, CROUND,
                                        ALU.add, ALU.subtract)
                scb = sc[:][:, :, None].to_broadcast((P, NBW, 64))
                wq3 = wq[:].rearrange("p (b e) -> p b e", e=64)
                nc.vector.tensor_tensor(wq3, wq3, scb, ALU.mult)
                # regroup k-major (contiguous per k), then per-offset lhsT
                # tiles w[k][i, o] = wq[o, i*9+k] via the DMA crossbar
                wq_r = setup.tile([P, WLEN], BF16, tag=f"wqr{wi}")
                nc.vector.tensor_copy(
                    wq_r[:].rearrange("p (k i) -> p k i", k=9),
                    wq[:].rearrange("p (i k) -> p k i", k=9))
                wk = []
                for k in range(9):
                    wt = const.tile([P, P], BF16, tag=f"w{wi}k{k}")
                    nc.sync.dma_start_transpose(wt[:], wq_r[:, k * P:(k + 1) * P])
                    wk.append(wt)
                return wk

            def _setup_bn():
                invb = []
                for i in ("1", "2"):
                    s = setup.tile([P, 1], F32, tag=f"sd{i}")
                    nc.scalar.activation(s[:], bnc[f"var{i}"][:], ACTF.Sqrt,
                                         bias=eps_b[:])
                    r = setup.tile([P, 1], F32, tag=f"rs{i}")
                    nc.vector.reciprocal(r[:], s[:])
                    inv = const.tile([P, 1], F32, tag=f"inv{i}")
                    nc.vector.tensor_tensor(inv[:], bnc[f"gamma{i}"][:], r[:],
                                            ALU.mult)
                    mi = setup.tile([P, 1], F32, tag=f"mi{i}")
                    nc.vector.tensor_tensor(mi[:], bnc[f"mean{i}"][:], inv[:],
                                            ALU.mult)
                    b = const.tile([P, 1], F32, tag=f"b{i}")
                    nc.vector.tensor_tensor(b[:], bnc[f"beta{i}"][:], mi[:],
                                            ALU.subtract)
                    invb.append((inv, b))
                return invb

            psum1_p = ctx.enter_context(
                tc.tile_pool(name="psum1", bufs=4, space="PSUM"))
            psum2_p = ctx.enter_context(
                tc.tile_pool(name="psum2", bufs=4, space="PSUM"))

            mids = [None] * nimg
            t2s = [None] * nimg

            def _quant_image(src, pad, tagq, nm):
                """BFP-quantize src (f32 [P,3136]) into pad's interior.
                V: reduce/smalls/mult/clip/round; G: scale-back (halved);
                S: strided insert into the padded layout."""
                src3 = src[:].rearrange("p (b e) -> p b e", e=64)
                rb, sc = _emit_scale_bits(src3, NBX, tagq)
                t = tmp_p.tile([P, HWF], F32, tag="t", name=f"t_{nm}")
                t3 = t[:].rearrange("p (b e) -> p b e", e=64)
                rsc = rb[:].bitcast(F32)[:, :, None].to_broadcast((P, NBX, 64))
                nc.vector.tensor_tensor(t3, src3, rsc, ALU.mult)
                nc.vector.tensor_scalar(t[:], t[:], CLIPV, -CLIPV,
                                        ALU.min, ALU.max)
                u = u_p.tile([P, HWF], BF16, tag="u", name=f"u_{nm}")
                nc.vector.tensor_scalar(u[:], t[:], CROUND, CROUND,
                                        ALU.add, ALU.subtract)
                scb = sc[:][:, :, None].to_broadcast((P, NBX, 64))
                u3 = u[:].rearrange("p (b e) -> p b e", e=64)
                for b0, b1 in ((0, HB), (HB, NBX)):
                    nc.gpsimd.tensor_tensor(u3[:, b0:b1], u3[:, b0:b1],
                                            scb[:, b0:b1], ALU.mult)
                nc.scalar.copy(_interior(pad),
                               u[:].rearrange("p (h w) -> p h w", w=W))

            def quant1(n):
                _quant_image(xraws[n], xq_pads[n % 2], "q1", f"q1_{n}")

            def quant2(n):
                _quant_image(mids[n], mq_pads[n % 2], "q2", f"q2_{n}")

            def _emit_conv(psum_pool, wk, src_pad, evict):
                for c in range(NCHUNK):
                    h0 = c * 8
                    ps = psum_pool.tile([P, CHF], F32, tag="pschunk")
                    for k in range(9):
                        kh, kw = divmod(k, 3)
                        s = (h0 + kh) * PITCH + kw
                        nc.tensor.matmul(
                            ps[:], wk[k][:], src_pad[:, s : s + CHF],
                            start=(k == 0), stop=(k == 8),
                        )
                    evict(c, ps)

            def conv1(n):
                mid = mid_p.tile([P, HWF], F32, tag="mid", name=f"mid{n}")
                mids[n] = mid

                def evict1(c, ps):
                    psv = ps[:].rearrange("p (r w) -> p r w", w=PITCH)[:, :, 1:1 + W]
                    ov = mid[:, c * 448:(c + 1) * 448].rearrange(
                        "p (r w) -> p r w", w=W)
                    nc.scalar.activation(ov, psv, ACTF.Relu,
                                         bias=b1[:], scale=inv1[:])

                _emit_conv(psum1_p, w1k, xq_pads[n % 2][:], evict1)

            def conv2(n, fuse_final=False):
                t2 = t2_p.tile([P, HWF], F32, tag="t2", name=f"t2_{n}")
                t2s[n] = t2

                def evict2(c, ps):
                    psv = ps[:].rearrange("p (r w) -> p r w", w=PITCH)[:, :, 1:1 + W]
                    sl = slice(c * 448, (c + 1) * 448)
                    ov = t2[:, sl].rearrange("p (r w) -> p r w", w=W)
                    nc.scalar.activation(ov, psv, ACTF.Identity,
                                         bias=b2[:], scale=inv2[:])
                    if fuse_final:
                        nc.gpsimd.tensor_tensor(t2[:, sl], t2[:, sl],
                                                xraws[n][:, sl], ALU.add)
                        nc.scalar.activation(t2[:, sl], t2[:, sl], ACTF.Relu,
                                             bias=zero_b[:])
                        nc.sync.dma_start(
                            out_d[n].rearrange("c h w -> c (h w)")[:, sl],
                            t2[:, sl])

                _emit_conv(psum2_p, w2k, mq_pads[n % 2][:], evict2)

            def final(n):
                """residual add on GpSimd (halved), relu on ScalarE, store."""
                t2 = t2s[n]
                h = HWF // 2
                for sl in (slice(0, h), slice(h, HWF)):
                    nc.gpsimd.tensor_tensor(t2[:, sl], t2[:, sl],
                                            xraws[n][:, sl], ALU.add)
                nc.scalar.activation(t2[:], t2[:], ACTF.Relu, bias=zero_b[:])
                nc.sync.dma_start(out_d[n].rearrange("c h w -> c (h w)"), t2[:])

            # ---- software pipeline ----
            w1k = _setup_w(0)
            (inv1, b1), (inv2, b2) = _setup_bn()
            quant1(0)
            w2k = _setup_w(1)
            quant1(1)
            conv1(0)
            quant1(2)   # after conv1(0): overwrites xq_pads[0]
            quant2(0)
            conv1(1)
            for k in range(nimg):
                conv2(k, fuse_final=(k == nimg - 1))
                if k + 1 < nimg:
                    quant2(k + 1)
                if k >= 1:
                    final(k - 1)
                if k + 2 < nimg:
                    conv1(k + 2)
                if k + 3 < nimg:
                    quant1(k + 3)
                if k + 4 < nimg:
                    load_x(k + 4)

    nc.compile()
    return nc


@lru_cache(maxsize=1)
def _get_nc():
    return build_nc(NIMG)


def kernel(x, w1, w2, gamma1, beta1, mean1, var1,
           gamma2, beta2, mean2, var2, _trace=False):
    f = lambda a: np.ascontiguousarray(np.asarray(a, dtype=np.float32))
    x = f(x)
    n_total = x.shape[0]
    assert n_total == N_CORES * NIMG, x.shape
    xs = x.reshape(N_CORES, NIMG, P, H, W)
    rep = {
        "w1": f(w1), "w2": f(w2),
        "gamma1": f(gamma1), "beta1": f(beta1), "mean1": f(mean1), "var1": f(var1),
        "gamma2": f(gamma2), "beta2": f(beta2), "mean2": f(mean2), "var2": f(var2),
    }
    in_maps = [{"x": np.ascontiguousarray(xs[c]), **rep} for c in range(N_CORES)]
    nc = _get_nc()
    res = run_bass_kernel_spmd(nc, in_maps, core_ids=list(range(N_CORES)),
                               trace=_trace)
    out = np.concatenate([res.results[c]["out"] for c in range(N_CORES)], axis=0)
    if _trace:
        kernel.last_result = res
    return out.reshape(n_total, P, H, W)
